# revision 9
# baseline (speedup 1.0000x reference)
"""Multi-head causal attention (B=2,S=2048,E=1024,H=16) on 8 TRN2 cores.

Sharding: core c -> batch b=c//4, head-group g=c%4 (heads 4g..4g+3).
Each core computes qkv for its 4 heads, causal attention, and the partial
output projection (its heads' columns of w_proj); host sums the 4 partials
per batch and adds the bias.

On-chip layout (per core, fp32r matmuls at full PE rate):
  xT   [E,S]   x[b].T
  qT/kT stored head-pair-stacked: Q01/K01/Q23/K23 [128, S]
       (partitions 0-63 head 2p, 64-127 head 2p+1 -> 2-head packed
        score matmuls via row groups)
  V    [S=16x128, 4*65] per k-tile, per head [V(64) | ones] -> the PV
       matmul emits attention output AND softmax denominators (row 64)
  scores computed transposed St=[k,q] so PV needs no transposes; causal
       handled by skipping k-tiles above the diagonal, restricting the
       q-subrange per diagonal tile, and a triangular -1e30 mask add
"""
import numpy as np

import concourse.bass as bass
from concourse import bacc
import concourse.mybir as mybir
import concourse.tile as tile
from concourse.bass_utils import run_bass_kernel_spmd

B, S, E, H = 2, 2048, 1024, 16
HD = 64
F32 = mybir.dt.float32
F32R = mybir.dt.float32r
NEG = -1.0e30
NQ = S // 512   # q-chunks
NK = S // 128   # k-tiles
NE = E // 128   # e-tiles


def build(debug=False):
    nc = bacc.Bacc("TRN2", target_bir_lowering=False, debug=False)
    xT = nc.declare_dram_parameter("xT", [E, S], F32R, isOutput=False)
    wq = nc.declare_dram_parameter("wq", [E, 512], F32R, isOutput=False)
    wv = nc.declare_dram_parameter("wv", [E, 256], F32R, isOutput=False)
    wp = nc.declare_dram_parameter("wp", [4, HD, E], F32R, isOutput=False)
    msk = nc.declare_dram_parameter("msk", [128, 384], F32, isOutput=False)
    one = nc.declare_dram_parameter("one", [128, 1], F32R, isOutput=False)
    y = nc.declare_dram_parameter("y", [S, E], F32, isOutput=True)
    if debug:
        dbg_qk = nc.declare_dram_parameter("dbg_qk", [128, S], F32, isOutput=True)
        dbg_v = nc.declare_dram_parameter("dbg_v", [128, 4 * (HD + 1)], F32, isOutput=True)
        dbg_O = nc.declare_dram_parameter("dbg_O", [HD + 1, 512], F32, isOutput=True)
        dbg_o = nc.declare_dram_parameter("dbg_o", [HD, S], F32, isOutput=True)

    with tile.TileContext(nc) as tc:
        with (
            tc.tile_pool(name="big", bufs=1) as big,
            tc.tile_pool(name="ps", bufs=2, space="PSUM") as ps,
            tc.tile_pool(name="pso", bufs=4, space="PSUM") as pso,
            tc.tile_pool(name="work", bufs=3) as work,
            tc.tile_pool(name="nrm", bufs=2) as nrm,
        ):
            msk_sb = big.tile([128, 384], F32, tag="msk")
            nc.sync.dma_start(out=msk_sb, in_=msk[:, :])
            tri = msk_sb[:, 0:128]
            maskhi = msk_sb[:, 128:384]

            wq_sb = big.tile([128, NE, 512], F32R, tag="wq")
            nc.sync.dma_start(out=wq_sb, in_=wq.rearrange("(t p) m -> p t m", p=128))
            wv_sb = big.tile([128, NE, 256], F32R, tag="wv")
            nc.sync.dma_start(out=wv_sb, in_=wv.rearrange("(t p) m -> p t m", p=128))
            wp_sb = big.tile([HD, 4, E], F32R, tag="wp")
            nc.sync.dma_start(out=wp_sb, in_=wp.rearrange("h p n -> p h n"))

            # x^T in 4 column chunks of 512 so compute can start early
            xc = []
            for sc in range(NQ):
                t = big.tile([128, NE, 512], F32R, tag=f"x{sc}")
                nc.sync.dma_start(
                    out=t,
                    in_=xT.rearrange("(t p) s -> p t s", p=128)[:, :, sc * 512:(sc + 1) * 512])
                xc.append(t)

            qk = [big.tile([128, S], F32R, tag=f"qk{i}", name=f"qk{i}") for i in range(4)]
            v_sb = big.tile([128, NK, 4 * (HD + 1)], F32R, tag="v")
            v4 = v_sb.rearrange("p s (h c) -> p s h c", h=4)
            one_ap = one[:, :]
            one_b = bass.AP(tensor=one_ap.tensor, offset=one_ap.offset,
                            ap=[one_ap.ap[0], [0, NK * 4], one_ap.ap[1]])
            v_ones = v_sb.rearrange("p s (h c) -> p (s h) c", c=HD + 1)[:, :, HD:HD + 1]
            nc.sync.dma_start(out=v_ones, in_=one_b)
            o_sb = [big.tile([HD, S], F32R, tag=f"o{h}", name=f"o{h}") for h in range(4)]

            # ---- phase A: q/k projections  qk[m][:, sc] = (Wsel @ x^T) ----
            for m in range(4):
                for sc in range(NQ):
                    acc = ps.tile([128, 512], F32, tag="pa")
                    for e in range(NE):
                        nc.tensor.matmul(
                            acc, lhsT=wq_sb[:, e, m * 128:(m + 1) * 128],
                            rhs=xc[sc][:, e, :],
                            start=(e == 0), stop=(e == NE - 1))
                    nc.vector.tensor_copy(qk[m][:, sc * 512:(sc + 1) * 512], acc)

            # ---- phase B: V = x @ Wv^T, [s,hd] per head with ones column ----
            for sb_i in range(NK):
                accv = ps.tile([128, 256], F32, tag="st")
                sc, blk = divmod(sb_i, 4)
                for e in range(NE):
                    nc.tensor.matmul(
                        accv, lhsT=xc[sc][:, e, blk * 128:(blk + 1) * 128],
                        rhs=wv_sb[:, e, :],
                        start=(e == 0), stop=(e == NE - 1))
                nc.vector.tensor_copy(
                    v4[:, sb_i, :, 0:HD],
                    accv.rearrange("p (h c) -> p h c", h=4))

            # ---- phase C: causal attention, head pairs ----
            for p in range(2):
                Q, K = qk[2 * p], qk[2 * p + 1]
                for qc in range(NQ):
                    O = [pso.tile([HD + 1, 512], F32, tag="O", name=f"O{p}_{qc}_{i}") for i in range(2)]
                    last = 4 * qc + 3
                    for ki in range(4 * qc + 4):
                        diag = ki >= 4 * qc
                        off = (ki - 4 * qc) * 128 if diag else 0
                        o_ = min(off, 256)
                        w = 512 - o_
                        for hh in range(2):
                            bp = 64 * hh
                            st = ps.tile([128, 512], F32, tag="st")
                            nc.tensor.matmul(
                                st[:, 0:w],
                                lhsT=K[bp:bp + 64, ki * 128:(ki + 1) * 128],
                                rhs=Q[bp:bp + 64, qc * 512 + o_:(qc + 1) * 512],
                                start=True, stop=True)
                            if diag:
                                if off == 384:
                                    nc.vector.tensor_add(st[:, 0:256], st[:, 0:256], maskhi)
                                else:
                                    nc.vector.tensor_add(st[:, 0:128], st[:, 0:128], tri)
                            pt = work.tile([128, 512], F32R, tag="pt")
                            nc.scalar.activation(
                                pt[:, 0:w], st[:, 0:w],
                                mybir.ActivationFunctionType.Exp, scale=0.125)
                            h = 2 * p + hh
                            nc.tensor.matmul(
                                O[hh][:, o_:512],
                                lhsT=v_sb[:, ki, h * 65:h * 65 + 65],
                                rhs=pt[:, 0:w],
                                start=(ki == 0), stop=(ki == last))
                    if debug and p == 0 and qc == 0:
                        dbgt = work.tile([HD + 1, 512], F32, tag="dbgO", bufs=1)
                        nc.vector.tensor_copy(dbgt, O[0])
                        nc.sync.dma_start(out=dbg_O[:, :], in_=dbgt)
                    for hh in range(2):
                        h = 2 * p + hh
                        rc = nrm.tile([1, 512], F32, tag="rc")
                        nc.vector.reciprocal(rc[0:1, :], O[hh][HD:HD + 1, :])
                        bc = nrm.tile([HD, 512], F32, tag="bc")
                        nc.gpsimd.partition_broadcast(bc, rc[0:1, :])
                        nc.vector.tensor_mul(
                            o_sb[h][:, qc * 512:(qc + 1) * 512], O[hh][0:HD, :], bc)

            if debug:
                nc.sync.dma_start(out=dbg_qk[:, :], in_=qk[0][:, :].bitcast(F32))
                nc.sync.dma_start(out=dbg_v[:, :], in_=v_sb[:, 0, :].bitcast(F32))
                nc.sync.dma_start(out=dbg_o[:, :], in_=o_sb[0][:, :].bitcast(F32))

            # ---- phase D: partial out-projection ----
            for qi in range(NK):
                for nch in range(2):
                    accy = ps.tile([128, 512], F32, tag="pa")
                    for h in range(4):
                        nc.tensor.matmul(
                            accy, lhsT=o_sb[h][:, qi * 128:(qi + 1) * 128],
                            rhs=wp_sb[:, h, nch * 512:(nch + 1) * 512],
                            start=(h == 0), stop=(h == 3))
                    yt = work.tile([128, 512], F32, tag="yt")
                    nc.vector.tensor_copy(yt, accy)
                    nc.sync.dma_start(
                        out=y[qi * 128:(qi + 1) * 128, nch * 512:(nch + 1) * 512],
                        in_=yt)
    nc.compile()
    return nc


_NC = None


def _get_nc():
    global _NC
    if _NC is None:
        _NC = build()
    return _NC


def _make_masks():
    r = np.arange(128)[:, None]
    j = np.arange(128)[None, :]
    tri = np.where(j >= r, 0.0, NEG).astype(np.float32)
    jh = np.arange(256)[None, :]
    maskhi = np.where(jh >= 128 + r, 0.0, NEG).astype(np.float32)
    return np.ascontiguousarray(np.concatenate([tri, maskhi], axis=1))


def _in_maps(x, w_qkv, w_proj):
    msk = _make_masks()
    xTb = [np.ascontiguousarray(x[b].T) for b in range(B)]
    maps = []
    for c in range(8):
        b, g = divmod(c, 4)
        hs = [4 * g + i for i in range(4)]
        rows = []
        for pair in range(2):
            for qk_sel in range(2):  # 0 -> q rows, 1 -> k rows
                for h in (hs[2 * pair], hs[2 * pair + 1]):
                    base = h * 3 * HD + qk_sel * HD
                    rows.append(w_qkv[base:base + HD])
        wq = np.ascontiguousarray(np.concatenate(rows, axis=0).T)
        vrows = [w_qkv[h * 3 * HD + 2 * HD:h * 3 * HD + 3 * HD] for h in hs]
        wv = np.ascontiguousarray(np.concatenate(vrows, axis=0).T)
        wp = np.ascontiguousarray(
            np.stack([w_proj[:, h * HD:(h + 1) * HD].T for h in hs], axis=0))
        maps.append({"xT": xTb[b], "wq": wq, "wv": wv, "wp": wp, "msk": msk,
                     "one": np.ones((128, 1), dtype=np.float32)})
    return maps


def run(inputs, trace=False):
    nc = _get_nc()
    maps = _in_maps(inputs["x"], inputs["w_qkv"], inputs["w_proj"])
    res = run_bass_kernel_spmd(nc, maps, core_ids=list(range(8)), trace=trace)
    b_proj = inputs["b_proj"]
    y = np.zeros((B, S, E), dtype=np.float32)
    for c in range(8):
        y[c // 4] += res.results[c]["y"]
    y += b_proj[None, None, :].astype(np.float32)
    return y, res


def kernel(x, w_qkv, w_proj, b_proj):
    y, _ = run({"x": np.asarray(x, dtype=np.float32),
                "w_qkv": np.asarray(w_qkv, dtype=np.float32),
                "w_proj": np.asarray(w_proj, dtype=np.float32),
                "b_proj": np.asarray(b_proj, dtype=np.float32)})
    return y
